# revision 32
# baseline (speedup 1.0000x reference)
"""Bidirectional attention kernel for Trainium2 (8 NeuronCores, batch-parallel).

Math (per batch element, all on one core):
    k1p = k1 @ W1 + b1            [N, A]
    k2p = k2 @ W2 + b2            [N, A]
    S   = k1p @ k2p.T             [N, N]
    E   = exp(S)                  (no max-subtraction needed: |S| < ~25)
    o1[m, d] = sum_n E[n, m] v1[n, d] / sum_n E[n, m]   (softmax over N1)
    o2[n, d] = sum_m E[n, m] v2[m, d] / sum_m E[n, m]   (softmax over N2)

Schedule (v4 — restructured prologue + just-in-time DMA feed; v3 was 97.9us
TimelineSim / 99.5us HW):
  * All input DMAs are issued in priority order decoupled from the compute
    that consumes them: k1c0, k2c0, k2c1 (prologue), then v2/k1c2/k1c3/v1
    chunks and the 16 ET slab transposes spliced between score strips, so
    the single 350GB/s DMA pipe is never holding up the next PE/ACT op.
  * Rows 0-3 run h0 strips first (needing only k2 chunks 0-1), then h1
    strips, with the remaining k-chunk preps spliced between strips; row 0
    h0 is split into 2x512 strips so the first exp starts right after k2c0.
  * ET (for o2) is produced by 16 DMA xbar slab transposes as in v3.
  * Inline o2 runs as a budgeted js-stream with partial-j groups: a tile
    opens as soon as its slab is ~1 row old and advances only as far as the
    v2 chunks that have been copied, so PE backfills the ACT-paced score
    loop without ever queueing behind a distant DMA.
  * Phase C: o1 groups (E is complete, zero waits) then the remaining o2
    groups; output tiles normalize on DVE and store in 4-tile batches, the
    last two singly.
"""

import numpy as np

import concourse.bass as bass
import concourse.tile as tile
from concourse import bacc, mybir, bass_utils
from concourse.masks import make_identity

N_CORES = 8
B = 8
N = 2048  # N1 == N2
KD = 256  # K1D == K2D
VD = 256  # V1D == V2D
AD = 128
P = 128

F32 = mybir.dt.float32
F32R = mybir.dt.float32r
BF16 = mybir.dt.bfloat16
AF = mybir.ActivationFunctionType

ROW_JS = 13   # inline o2 matmul budget per score row
SLAB_LAG = 7  # rows between slab issue and first consumption: sync-ring DMA
# completion lanes recycle as consumers drain, so consumption must chase
# issuance with enough slack for the serialized DMA pipe to deliver


class _OutBatcher:
    """Accumulates normalized output tiles and stores them 4-at-a-time with a
    single DMA (HWDGE dispatch overhead is ~625ns; 8 big stores beat 32).

    Stores are emitted with one batch of lag: batch N's DMA goes on the sync
    ring only once batch N+1 is full, so its data (DVE normalizes) is long
    done by the time it reaches the in-order ring head and never blocks the
    ET slab transposes queued behind it.  All traffic stays on HWDGE: bulky
    SWDGE transfers would occupy the same physical DMA queues the transposes
    are assigned to and serialize them (queue-order semaphores)."""

    def __init__(self, nc, osb_pool, group=4, defer=True):
        self.nc = nc
        self.osb_pool = osb_pool
        self.group = group
        self.defer = defer
        self.cur = {}  # o_d id -> (tile, base_mt, count, o_d)
        self.full = {}  # o_d id -> deferred full batch entry

    def _store(self, entry):
        tile_, base, cnt, o_d = entry
        self.nc.sync.dma_start(
            out=o_d[128 * base : 128 * (base + cnt), :].rearrange(
                "(t p) d -> p t d", p=P
            ),
            in_=tile_[:, 0:cnt, :],
        )

    def slot(self, o_d, mt):
        key = id(o_d)
        tile_, base, cnt, _ = self.cur.get(key, (None, None, 0, None))
        if tile_ is None or cnt == self.group or mt != base + cnt:
            self._retire(key)
            tile_ = self.osb_pool.tile([P, self.group, VD], F32, tag="ob", name="ob")
            self.cur[key] = (tile_, mt, 1, o_d)
            return tile_[:, 0, :]
        self.cur[key] = (tile_, base, cnt + 1, o_d)
        return tile_[:, cnt, :]

    def _retire(self, key):
        entry = self.cur.pop(key, None)
        if entry is None or entry[0] is None:
            return
        if not self.defer:
            self._store(entry)
            return
        prev = self.full.pop(key, None)
        if prev is not None:
            self._store(prev)
        self.full[key] = entry

    def flush(self, key=None):
        if key is None:
            for k in list(set(self.cur) | set(self.full)):
                self.flush(k)
            return
        self._retire(key)
        prev = self.full.pop(key, None)
        if prev is not None:
            self._store(prev)


def _emit_o_mms(nc, pot, Esrc, ve, mt, j0, j1, nt):
    for j in range(j0, j1):
        nc.tensor.matmul(
            pot,
            lhsT=Esrc[:, j, 128 * mt : 128 * (mt + 1)],
            rhs=ve[:, j, 0 : VD + 1],
            start=(j == 0),
            stop=(j == nt - 1),
        )


def _emit_o_norm(nc, rc_pool, ob_batch, pot, o_d, mt):
    rc = rc_pool.tile([P, 1], F32, tag="rc", name="rct")
    nc.vector.reciprocal(rc, pot[:, VD : VD + 1])
    ob = ob_batch.slot(o_d, mt)
    nc.vector.tensor_scalar_mul(ob, pot[:, 0:VD], rc)


def _emit_o_group(nc, po_pool, rc_pool, ob_batch, Esrc, ve, o_d, mt, nt):
    """One output tile: 16-deep PSUM accumulation + folded-softmax normalize."""
    pot = po_pool.tile([P, VD + 1], F32, tag="po", name="pot")
    _emit_o_mms(nc, pot, Esrc, ve, mt, 0, nt, nt)
    _emit_o_norm(nc, rc_pool, ob_batch, pot, o_d, mt)


def _emit_body(nc, tc, consts, persist, dram, n, pools, warmup=True):
    """One full pass (v4 schedule). n must be 2048 (nt=16, nch=4)."""
    nt = n // P
    nch = n // 512
    assert nch == 4 and nt == 16, "v4 schedule is specialized to N=2048"
    k1_d, k2_d, v1_d, v2_d, o1_d, o2_d = (
        dram["k1"], dram["k2"], dram["v1"], dram["v2"], dram["o1"], dram["o2"],
    )
    identity, W1_sb, b1_sb, W2_sb, b2_sb = consts

    k1pT = persist.tile([P, n], F32R, tag="k1pT", name="k1pT")
    k2pT = persist.tile([P, n], F32R, tag="k2pT", name="k2pT")
    E = persist.tile([P, nt, n], BF16, tag="E", name="E")
    ET = persist.tile([P, nt, n], BF16, tag="ET", name="ET")
    v1e = persist.tile([P, nt, VD + 2], BF16, tag="v1e", name="v1e")
    v2e = persist.tile([P, nt, VD + 2], BF16, tag="v2e", name="v2e")

    stage, ktbuf = pools["stage"], pools["ktbuf"]
    osb_pool, rc_pool = pools["osb"], pools["rc"]

    # PSUM (8 banks): pscore 2x[P,1024] = 4 + ptpp 3x[P,512] = 3 during the
    # prologue (3 bufs break the circular WAR knot in the prep chains);
    # ptpp closes after the last k-prep -> po0 (2) + po1 (2).  pscore closes
    # after the score loop -> po2 (4).
    pscore_cm = tc.tile_pool(name="pscore", bufs=2, space="PSUM")
    pscore = pscore_cm.__enter__()
    ptpp_cm = tc.tile_pool(name="ptpp", bufs=3, space="PSUM")
    ptpp = ptpp_cm.__enter__()
    po0_cm = None
    po0 = None
    po_pools = []
    n_groups = 0

    # ---- helpers -------------------------------------------------------
    def load_chunk(t_d, c, dim=KD, eng="sync"):
        st = stage.tile([P, 4, dim], F32, tag="stage", name="st")
        getattr(nc, eng).dma_start(
            out=st,
            in_=t_d[512 * c : 512 * (c + 1), :].rearrange("(t p) k -> p t k", p=P),
        )
        return st

    def prep_chunk(st, W_sb, b_sb, kpT, c, copy_eng="vector"):
        ceng = getattr(nc, copy_eng)
        kt = ktbuf.tile([P, 2, 512], F32R, tag="kt", name="kt")
        for kb in range(2):
            pt = ptpp.tile([P, 512], F32, tag="pt512", name="pt")
            for t in range(4):
                nc.tensor.transpose(
                    pt[:, 128 * t : 128 * (t + 1)],
                    st[:, t, 128 * kb : 128 * (kb + 1)],
                    identity,
                )
            if copy_eng == "scalar":
                ceng.activation(kt[:, kb, :], pt, AF.Identity)
            else:
                ceng.tensor_copy(kt[:, kb, :], pt)
        pp = ptpp.tile([P, 512], F32, tag="pt512", name="pp")
        for kb in range(2):
            nc.tensor.matmul(
                pp, lhsT=W_sb[:, kb, :], rhs=kt[:, kb, :],
                start=(kb == 0), stop=(kb == 1),
            )
        nc.vector.tensor_scalar_add(kpT[:, 512 * c : 512 * (c + 1)], pp, b_sb)

    def prep_half(st, W_sb, b_sb, kpT, c, hf):
        kt = ktbuf.tile([P, 2, 512], F32R, tag="kt", name="kt")
        for kb in range(2):
            pt = ptpp.tile([P, 512], F32, tag="pt512", name="pt")
            for t in range(2):
                nc.tensor.transpose(
                    pt[:, 128 * t : 128 * (t + 1)],
                    st[:, 2 * hf + t, 128 * kb : 128 * (kb + 1)],
                    identity,
                )
            nc.vector.tensor_copy(kt[:, kb, 0:256], pt[:, 0:256])
        pp = ptpp.tile([P, 512], F32, tag="pt512", name="pp")
        for kb in range(2):
            nc.tensor.matmul(
                pp[:, 0:256], lhsT=W_sb[:, kb, :], rhs=kt[:, kb, 0:256],
                start=(kb == 0), stop=(kb == 1),
            )
        nc.vector.tensor_scalar_add(
            kpT[:, 512 * c + 256 * hf : 512 * c + 256 * (hf + 1)],
            pp[:, 0:256], b_sb,
        )

    def copy_vchunk(sv, ve, c, eng="gpsimd"):
        getattr(nc, eng).tensor_copy(ve[:, 4 * c : 4 * (c + 1), 0:VD], sv)

    def emit_strip_q(i, h, q, ps):
        col = 1024 * h + 512 * q
        nc.tensor.matmul(
            ps[:, 512 * q : 512 * (q + 1)],
            lhsT=k1pT[:, 128 * i : 128 * (i + 1)],
            rhs=k2pT[:, col : col + 512],
            start=True,
            stop=True,
        )
        nc.scalar.activation(
            E[:, i, col : col + 512], ps[:, 512 * q : 512 * (q + 1)], AF.Exp
        )

    def emit_strip(i, h, split=False):
        ps = pscore.tile([P, 1024], F32, tag="ps", name="ps")
        if split:
            for q in range(2):
                emit_strip_q(i, h, q, ps)
            return
        for q in range(2):
            col = 1024 * h + 512 * q
            nc.tensor.matmul(
                ps[:, 512 * q : 512 * (q + 1)],
                lhsT=k1pT[:, 128 * i : 128 * (i + 1)],
                rhs=k2pT[:, col : col + 512],
                start=True,
                stop=True,
            )
        nc.scalar.activation(E[:, i, 1024 * h : 1024 * (h + 1)], ps, AF.Exp)

    slabs_done = 0

    def slab(i):
        nonlocal slabs_done
        nc.sync.dma_start_transpose(
            out=ET[:, :, 128 * i : 128 * (i + 1)], in_=E[:, i, :]
        )
        slabs_done += 1

    # ---- inline-o2 state machine --------------------------------------
    ob_batch = _OutBatcher(nc, osb_pool)
    o2_open = []  # [pot, mt, j_done]
    o2_next_tile = 0
    v2_chunks = 0
    max_open = 1

    def o2_drain(budget, open_new=True):
        nonlocal o2_next_tile, n_groups
        j_avail = 4 * v2_chunks
        avail_tiles = max(0, slabs_done - SLAB_LAG)
        while budget > 0:
            g = next((g for g in o2_open if g[2] < min(nt, j_avail)), None)
            if g is None:
                if (open_new and len(o2_open) < max_open
                        and o2_next_tile < avail_tiles and j_avail > 0):
                    pot = po_pools[n_groups % len(po_pools)].tile(
                        [P, VD + 1], F32, tag="po", name="pot"
                    )
                    n_groups += 1
                    g = [pot, o2_next_tile, 0]
                    o2_open.append(g)
                    o2_next_tile += 1
                else:
                    return
            pot, mt, j0 = g
            j1 = min(nt, j_avail, j0 + budget)
            _emit_o_mms(nc, pot, ET, v2e, mt, j0, j1, nt)
            budget -= j1 - j0
            g[2] = j1
            if j1 == nt:
                _emit_o_norm(nc, rc_pool, ob_batch, pot, o2_d, mt)
                o2_open.remove(g)

    # ---- prologue ------------------------------------------------------
    st_k1 = [None] * nch
    st_k2 = [None] * nch
    st_v2 = [None] * nch
    st_v1 = [None] * nch
    st_k1[0] = load_chunk(k1_d, 0)
    st_k2[0] = load_chunk(k2_d, 0)
    st_k2[1] = load_chunk(k2_d, 1)
    st_k2[2] = load_chunk(k2_d, 2)
    st_k2[3] = load_chunk(k2_d, 3)
    st_k1[1] = load_chunk(k1_d, 1)

    if warmup:
        # HAM warmup: dummy transposes on an *uninitialized* scratch tile
        # (no dependency on identity, which gpsimd builds at ~2.3us) keep
        # the PE busy from t=0 so the ~3.4us cold-clock window burns
        # throwaway work.  Results and data are garbage and discarded.
        wg = persist.tile([P, P], F32, tag="warmsrc", name="wg")
        nc.vector.memset(wg, 0.0)
        pwarm = ptpp.tile([P, P], F32, tag="pt512", name="warm")
        for _ in range(12):
            nc.tensor.matmul(pwarm[:, 0:P], lhsT=wg, rhs=wg, start=True,
                             stop=True)

    nc.gpsimd.memset(v2e[:, :, VD : VD + 2], 1.0)
    nc.gpsimd.memset(v1e[:, :, VD : VD + 2], 1.0)

    # all psum->sbuf copies ride DVE: ACT stays a pure exp engine (any ACT
    # copy would stall the exp stream behind it in ACT's in-order FIFO)
    prep_chunk(st_k1[0], W1_sb, b1_sb, k1pT, 0)
    prep_chunk(st_k2[0], W2_sb, b2_sb, k2pT, 0)
    # row 0 h0 split: the first 512-strip only needs k2 chunk 0; its second
    # half must wait for the c1 projection (reading k2pT cols 512-1023
    # before prep k2c1 is an unordered read of unwritten SBUF)
    ps00 = pscore.tile([P, 1024], F32, tag="ps", name="ps")
    emit_strip_q(0, 0, 0, ps00)
    st_v2[0] = load_chunk(v2_d, 0, VD)
    prep_chunk(st_k2[1], W2_sb, b2_sb, k2pT, 1)
    emit_strip_q(0, 0, 1, ps00)
    emit_strip(1, 0)
    st_v2[1] = load_chunk(v2_d, 1, VD)
    # all h0 strips before the k2c2/c3 preps: the preps wait on DMA and
    # would block ready strips behind them in the PE FIFO
    emit_strip(2, 0)
    emit_strip(3, 0)
    prep_chunk(st_k2[2], W2_sb, b2_sb, k2pT, 2)
    st_k1[2] = load_chunk(k1_d, 2)
    prep_chunk(st_k2[3], W2_sb, b2_sb, k2pT, 3)
    st_k1[3] = load_chunk(k1_d, 3)
    emit_strip(0, 1, split=True)
    st_v2[2] = load_chunk(v2_d, 2, VD)
    copy_vchunk(st_v2[0], v2e, 0)
    v2_chunks = 1
    emit_strip(1, 1, split=True)
    st_v2[3] = load_chunk(v2_d, 3, VD)
    copy_vchunk(st_v2[1], v2e, 1)
    v2_chunks = 2
    emit_strip(2, 1)
    emit_strip(3, 1)
    prep_chunk(st_k1[1], W1_sb, b1_sb, k1pT, 1)
    # all input loads are now on the ring; slabs 0-3 follow them so the
    # DMA pipe runs loads back-to-back, then slabs back-to-back
    slab(0)
    slab(1)
    slab(2)
    slab(3)

    # ---- steady rows 4..15 --------------------------------------------
    po1_cm = None
    for i in range(4, nt):
        if i == 4:
            copy_vchunk(st_v2[2], v2e, 2)
            v2_chunks = 3
        if i == 5:
            copy_vchunk(st_v2[3], v2e, 3)
            v2_chunks = 4
        if i == 6:
            prep_chunk(st_k1[2], W1_sb, b1_sb, k1pT, 2)
        if i == 7:
            prep_chunk(st_k1[3], W1_sb, b1_sb, k1pT, 3)
            # last k-prep done: free the 3 ptpp banks for the o2 pools
            ptpp_cm.__exit__(None, None, None)
            po0_cm = tc.tile_pool(name="po0", bufs=2, space="PSUM")
            po0 = po0_cm.__enter__()
            po_pools.append(po0)
            po1_cm = tc.tile_pool(name="po1", bufs=2, space="PSUM")
            po_pools.append(po1_cm.__enter__())
        emit_strip(i, 0)
        emit_strip(i, 1)
        slab(i)
        o2_drain(ROW_JS)

    # v1 loads ride the ring only now, behind the last slabs: during the
    # loop the pipe belongs to the slabs (leftover-o2 runs first in phase C
    # and covers v1's flight; o1 follows once v1e is up)
    for c in range(nch):
        st_v1[c] = load_chunk(v1_d, c, VD)
    for c in range(nch):
        copy_vchunk(st_v1[c], v1e, c, eng="vector")
    # finish the open partial groups only
    v2_chunks = nch
    o2_drain(10**9, open_new=False)

    # ---- phase C: o1 burst, then o2 leftovers -------------------------
    def emit_group(Esrc, ve, o_d, mt):
        nonlocal n_groups
        _emit_o_group(nc, po_pools[n_groups % len(po_pools)], rc_pool, ob_batch,
                      Esrc, ve, o_d, mt, nt)
        n_groups += 1

    # leftover o2 groups first (their slabs just landed; v1 is still in
    # flight), two on the old rotation so the pscore/po1 release barriers
    # overlap real PE work, then reopen as po1b + a deeper po2
    leftover = list(range(o2_next_tile, nt))
    for mt in leftover[:2]:
        emit_group(ET, v2e, o2_d, mt)
    po1_cm.__exit__(None, None, None)
    po0_cm.__exit__(None, None, None)
    pscore_cm.__exit__(None, None, None)
    po0_cm = tc.tile_pool(name="po0b", bufs=2, space="PSUM")
    po1_cm = tc.tile_pool(name="po1b", bufs=2, space="PSUM")
    po2_cm = tc.tile_pool(name="po2", bufs=4, space="PSUM")
    po_pools = [po0_cm.__enter__(), po2_cm.__enter__(), po1_cm.__enter__()]
    for mt in leftover[2:]:
        emit_group(ET, v2e, o2_d, mt)
    # o1 burst; last two tiles store as singles so the final DMA is small
    # and the end-of-program barrier isn't gated on a 4-tile store
    tail_batch = _OutBatcher(nc, osb_pool, group=1, defer=False)
    for mt in range(nt - 2):
        emit_group(E, v1e, o1_d, mt)
    ob_batch.flush()
    for mt in range(nt - 2, nt):
        _emit_o_group(nc, po_pools[n_groups % len(po_pools)], rc_pool,
                      tail_batch, E, v1e, o1_d, mt, nt)
        n_groups += 1
    tail_batch.flush()
    po1_cm.__exit__(None, None, None)
    po2_cm.__exit__(None, None, None)
    po0_cm.__exit__(None, None, None)


def _make_dram(nc, n):
    dram = {
        "k1": nc.dram_tensor("k1", [n, KD], F32, kind="ExternalInput").ap(),
        "k2": nc.dram_tensor("k2", [n, KD], F32, kind="ExternalInput").ap(),
        "v1": nc.dram_tensor("v1", [n, VD], F32, kind="ExternalInput").ap(),
        "v2": nc.dram_tensor("v2", [n, VD], F32, kind="ExternalInput").ap(),
        "o1": nc.dram_tensor("o1", [n, VD], F32, kind="ExternalOutput").ap(),
        "o2": nc.dram_tensor("o2", [n, VD], F32, kind="ExternalOutput").ap(),
    }
    W1_d = nc.dram_tensor("W1", [KD, AD], F32R, kind="ExternalInput").ap()
    b1_d = nc.dram_tensor("b1", [AD], F32, kind="ExternalInput").ap()
    W2_d = nc.dram_tensor("W2", [KD, AD], F32R, kind="ExternalInput").ap()
    b2_d = nc.dram_tensor("b2", [AD], F32, kind="ExternalInput").ap()
    return dram, (W1_d, b1_d, W2_d, b2_d)


def _load_consts(nc, consts_pool, wdram):
    W1_d, b1_d, W2_d, b2_d = wdram
    identity = consts_pool.tile([P, P], F32)
    # consts ride the sync ring at the very front (0.84us of transfers +
    # dispatch overhead before k1c0) so every W/b is resident before the
    # first projection; identity (gpsimd, no SWDGE gens ahead of it) is
    # ready by the first kprep transpose.
    make_identity(nc, identity)
    W1_sb = consts_pool.tile([P, 2, AD], F32R)
    nc.sync.dma_start(out=W1_sb, in_=W1_d.rearrange("(kb k) a -> k kb a", k=P))
    b1_sb = consts_pool.tile([P, 1], F32)
    nc.sync.dma_start(out=b1_sb, in_=b1_d.rearrange("(a one) -> a one", one=1))
    W2_sb = consts_pool.tile([P, 2, AD], F32R)
    nc.sync.dma_start(out=W2_sb, in_=W2_d.rearrange("(kb k) a -> k kb a", k=P))
    b2_sb = consts_pool.tile([P, 1], F32)
    nc.sync.dma_start(out=b2_sb, in_=b2_d.rearrange("(a one) -> a one", one=1))
    return (identity, W1_sb, b1_sb, W2_sb, b2_sb)


def _make_pools(tc, ctx):
    return {
        "stage": ctx.enter_context(tc.tile_pool(name="stage", bufs=6)),
        "ktbuf": ctx.enter_context(tc.tile_pool(name="ktbuf", bufs=2)),
        "osb": ctx.enter_context(tc.tile_pool(name="osb", bufs=3)),
        "rc": ctx.enter_context(tc.tile_pool(name="rc", bufs=4)),
    }


def build_nc(n: int = N, reps: int = 1):
    """Single-shot SPMD program (what kernel() runs)."""
    import contextlib

    nc = bacc.Bacc("TRN2", target_bir_lowering=False, debug=False)
    dram, wdram = _make_dram(nc, n)
    with tile.TileContext(nc) as tc:
        with tc.tile_pool(name="consts", bufs=1) as consts_pool, tc.tile_pool(
            name="persist", bufs=1
        ) as persist, contextlib.ExitStack() as ctx:
            consts = _load_consts(nc, consts_pool, wdram)
            pools = _make_pools(tc, ctx)
            for _ in range(reps):
                _emit_body(nc, tc, consts, persist, dram, n, pools)
    nc.compile()
    return nc


def build_nc_loop(n: int = N, iters: int = 16):
    """Timing variant: whole body inside a hardware For_i loop."""
    import contextlib

    nc = bacc.Bacc("TRN2", target_bir_lowering=False, debug=False)
    dram, wdram = _make_dram(nc, n)
    with tile.TileContext(nc) as tc:
        with tc.tile_pool(name="consts", bufs=1) as consts_pool, tc.tile_pool(
            name="persist", bufs=1
        ) as persist, contextlib.ExitStack() as ctx:
            consts = _load_consts(nc, consts_pool, wdram)
            pools = _make_pools(tc, ctx)
            with tc.For_i(0, iters, 1):
                _emit_body(nc, tc, consts, persist, dram, n, pools,
                           warmup=False)
    nc.compile()
    return nc


_NC_CACHE: dict = {}


def _get_nc(n: int = N):
    if n not in _NC_CACHE:
        _NC_CACHE[n] = build_nc(n)
    return _NC_CACHE[n]


def kernel(k1, k2, v1, v2, W1, b1, W2, b2):
    """Full-input entry point: shard batch across 8 cores, run SPMD, gather."""
    nc = _get_nc(N)
    k1 = np.ascontiguousarray(np.asarray(k1, dtype=np.float32))
    k2 = np.ascontiguousarray(np.asarray(k2, dtype=np.float32))
    v1 = np.ascontiguousarray(np.asarray(v1, dtype=np.float32))
    v2 = np.ascontiguousarray(np.asarray(v2, dtype=np.float32))
    W1 = np.ascontiguousarray(np.asarray(W1, dtype=np.float32))
    b1 = np.ascontiguousarray(np.asarray(b1, dtype=np.float32))
    W2 = np.ascontiguousarray(np.asarray(W2, dtype=np.float32))
    b2 = np.ascontiguousarray(np.asarray(b2, dtype=np.float32))
    in_maps = [
        {
            "k1": k1[c], "k2": k2[c], "v1": v1[c], "v2": v2[c],
            "W1": W1, "b1": b1, "W2": W2, "b2": b2,
        }
        for c in range(N_CORES)
    ]
    res = bass_utils.run_bass_kernel_spmd(nc, in_maps, core_ids=list(range(N_CORES)))
    o2 = np.stack([res.results[c]["o2"] for c in range(N_CORES)])
    o1 = np.stack([res.results[c]["o1"] for c in range(N_CORES)])
    return (o2, o1)
